# revision 10
# baseline (speedup 1.0000x reference)
"""MoE layer (B=8192, D=1024, E=8, top-2, H=2048) on 8 TRN2 NeuronCores.

Strategy (expert-parallel with two-segment load balancing):
  - Host: gate logits = x @ Wg (fp32), exact top-2 (jax tie-break semantics),
    softmax over the 2 picks. The 16384 (token, expert) pairs are grouped by
    expert and each expert's list is cut into pieces assigned to 16 fixed-size
    segments (8 of size S1, 8 of size S2, one of each per core) so every core
    carries ~the mean load instead of the max expert's load.
  - Device (SPMD): each core runs two segments; segment rows use that
    segment's expert weights: y = relu(x @ W1[e] + b1[e]) @ W2[e] in bf16
    with fp32 PSUM accumulation. Weights resident in SBUF, tokens processed
    in column chunks of <=512.
  - Host: weighted combine out[tok] = sum_k gate * (y + b2[e]).

The expert MLPs are the only O(B*D*H) work; gating/combine are O(B*D).
"""

import os

import numpy as np
import ml_dtypes

B, D, E, TOP_K = 8192, 1024, 8, 2
H = 2 * D
P = 128
CHUNK = 512

KD = D // P  # 8 contraction tiles for mm1 (over D)
MH = H // P  # 16 output tiles for mm1 / contraction tiles for mm2 (over H)
MD = D // P  # 8 output tiles for mm2 (over D)

_BF16 = np.dtype(ml_dtypes.bfloat16)

LAST_RESULTS = None  # BassKernelResults of the most recent run (for test harness)


def _chunk_sizes(n):
    """Split n columns into matmul chunks <=512, avoiding tiny tails."""
    chunks = [CHUNK] * (n // CHUNK)
    tail = n % CHUNK
    if tail >= 128 or not chunks:
        if tail:
            chunks.append(tail)
    elif tail:
        last = chunks.pop() + tail
        chunks += [last - last // 2, last // 2]
    return chunks


def _plan_segments(counts):
    """Pick segment sizes (S1 >= S2) and cut experts into 8 S1-pieces and
    8 S2-pieces (one of each per core). Returns (S1, S2, pieces) where
    pieces[core] = [(expert, tok_start, fill_len), (expert, tok_start, fill_len)]
    for the S1 and S2 segment respectively."""
    order = np.argsort(-counts, kind="stable")
    best = None
    for k in range(0, E // 2 + 1):
        big = order[:k]
        small = order[E - k :] if k else order[:0]
        mid = order[k : E - k]
        S1 = max((int(-(-counts[e] // 2)) for e in big), default=0)
        S2 = max((int(-(-counts[e] // 2)) for e in small), default=0)
        if len(mid):
            if S1 == 0:  # k == 0: all experts are (S1, S2)
                S1 = int(-(-counts[mid].max() // 2))
            S2 = max(S2, int(counts[mid].max()) - S1)
        C = S1 + S2
        if best is None or C < best[0]:
            best = (C, k, S1, S2)
    _, k, S1, S2 = best
    S1 = int(-(-S1 // 2) * 2)  # keep columns 4B-aligned in bf16
    S2 = int(-(-S2 // 2) * 2)

    big = order[:k]
    small = order[E - k :] if k else order[:0]
    mid = order[k : E - k]
    s1_pieces = []  # (expert, tok_start, len)
    s2_pieces = []
    for e in big:
        n = int(counts[e])
        h1 = -(-n // 2)
        s1_pieces += [(int(e), 0, h1), (int(e), h1, n - h1)]
    for e in small:
        n = int(counts[e])
        h1 = -(-n // 2)
        s2_pieces += [(int(e), 0, h1), (int(e), h1, n - h1)]
    for e in mid:
        n = int(counts[e])
        a = min(n, S1)
        s1_pieces.append((int(e), 0, a))
        s2_pieces.append((int(e), a, n - a))
    assert len(s1_pieces) == E and len(s2_pieces) == E
    assert all(ln <= S1 for _, _, ln in s1_pieces)
    assert all(ln <= S2 for _, _, ln in s2_pieces)
    pieces = [[s1_pieces[i], s2_pieces[i]] for i in range(E)]
    return S1, S2, pieces


def _build_program(S1, S2):
    import concourse.bacc as bacc
    import concourse.mybir as mybir
    import concourse.tile as tile
    from concourse.bass import ts

    C = S1 + S2
    nc = bacc.Bacc("TRN2", target_bir_lowering=False, debug=False)
    bf16 = mybir.dt.bfloat16
    f32 = mybir.dt.float32

    # DRAM layouts are host-packed for >=2KB contiguous DMA descriptor runs
    # (SDMA throughput is descriptor-size-bound: ~150GB/s @512B, ~390 @2KB):
    #   xt: chunk-major [p][ko][c-in-chunk] -> 8KB runs per partition
    #   w1: [m][p][ko][hl] -> 2KB runs per (partition, m-group)
    #   w2: natural (H, D) -> 2KB runs per (partition, k2-row)
    xt_d = nc.dram_tensor("xt", (P, KD * C), bf16, kind="ExternalInput").ap()
    w1_ds, w2_ds = [], []
    for s in ("a", "b"):
        w1_ds.append(
            nc.dram_tensor(f"w1{s}", (MH * P, KD * P), bf16, kind="ExternalInput").ap()
        )
        w2_ds.append(nc.dram_tensor(f"w2{s}", (H, D), bf16, kind="ExternalInput").ap())
    b1_d = nc.dram_tensor("b1ab", (P, 2 * MH), f32, kind="ExternalInput").ap()
    yt_d = nc.dram_tensor("yt", (D, C), bf16, kind="ExternalOutput").ap()

    chunk_list = []  # (seg, off, tw)
    off = 0
    for seg, seg_len in ((0, S1), (1, S2)):
        for tw in _chunk_sizes(seg_len):
            chunk_list.append((seg, off, tw))
            off += tw
    n_chunks = len(chunk_list)

    with tile.TileContext(nc) as tc:
        with (
            tc.tile_pool(name="weights", bufs=1) as wpool,
            tc.tile_pool(name="xin", bufs=1) as xpool,
            tc.tile_pool(name="hbuf", bufs=1) as hpool,
            tc.tile_pool(name="ystage", bufs=3) as ypool,
            tc.tile_pool(name="ps", bufs=8, space="PSUM") as pspool,
        ):
            xt_sbs = [
                xpool.tile([P, KD, tw], bf16, name=f"xt{ci}")
                for ci, (_, _, tw) in enumerate(chunk_list)
            ]
            b1_sb = wpool.tile([P, 2 * MH], f32, name="b1sb")
            w1_sbs, w2_sbs = [], []
            w1_rs, w2_rs = [], []
            for s in range(2):
                w1_sbs.append(wpool.tile([P, MH, KD, P], bf16, name=f"w1sb{s}"))
                w2_sbs.append(wpool.tile([P, MH, D], bf16, name=f"w2sb{s}"))
                w1_rs.append(w1_ds[s].rearrange("(m p) c -> p m c", p=P))
                w2_rs.append(w2_ds[s].rearrange("(ko p) d -> p ko d", p=P))

            # DMA plan: one HWDGE ring (sync) sustains ~390GB/s on its own
            # given >=2KB descriptors; the remaining constraints are issue
            # rate (~0.65us per dma_start) and need-order arrival.
            dma = nc.sync.dma_start
            dma(b1_sb, b1_d)  # 16KB: biases for both segments
            # w1a m-tile 0 (256KB) then xt chunk0 split by k-halves (2x512KB),
            # with w1a m1-2 between the halves: the (m0,k0..3) matmuls start
            # after the first xt piece, and m1's weights land before m0's
            # matmuls retire so the m-loop never starves during the ramp.
            dma(w1_sbs[0][:, 0], w1_rs[0][:, 0])
            c0_tw = chunk_list[0][2]
            dma(xt_sbs[0][:, 0:4], xt_d[:, 0 : 4 * c0_tw])
            dma(w1_sbs[0][:, 1:3], w1_rs[0][:, 1:3])
            dma(xt_sbs[0][:, 4:8], xt_d[:, 4 * c0_tw : KD * c0_tw])
            # rest of w1a in 512KB pieces (2 m-groups each)
            for m in range(3, MH, 2):
                m1 = min(m + 2, MH)
                dma(w1_sbs[0][:, m:m1], w1_rs[0][:, m:m1])
            # w2a in 512KB pieces (2 k2-rows each), consumed k2-progressively
            for k in range(0, MH, 2):
                dma(w2_sbs[0][:, k : k + 2], w2_rs[0][:, k : k + 2])
            # remaining xt chunks, one piece each (8KB runs)
            for ci in range(1, n_chunks):
                off = chunk_list[ci][1]
                tw = chunk_list[ci][2]
                dma(xt_sbs[ci], xt_d[:, KD * off : KD * (off + tw)])
            # segment B weights (needed ~120us in), 1MB pieces
            for m in range(0, MH, 4):
                dma(w1_sbs[1][:, m : m + 4], w1_rs[1][:, m : m + 4])
            for k in range(0, MH, 4):
                dma(w2_sbs[1][:, k : k + 4], w2_rs[1][:, k : k + 4])

            # Keep chunk 0 first (the DMA critical path is tuned for it) but
            # end on the smallest chunk so the final copy+DMA trail is short.
            order = list(range(n_chunks))
            tail_ci = min(order[1:], key=lambda ci: chunk_list[ci][2])
            order.remove(tail_ci)
            order.append(tail_ci)

            def mm2_phase(seg, off, tw, h_sb, k2_outer):
                # k2-outer: all 8 output banks accumulate together so each
                # w2[k2] slice is consumed as it lands (spreads the 4MB w2
                # demand over ~27us instead of needing it all upfront). The
                # last chunk uses m2-outer instead so its copies/output DMAs
                # overlap its own matmul stream rather than trailing it.
                w2_sb = w2_sbs[seg]
                if k2_outer:
                    # Two 4-bank halves: w2[k2] is still consumed
                    # progressively, but only 4 PSUM banks are held at a
                    # time and half-0's copies overlap half-1's matmuls.
                    for m2_base in (0, MD // 2):
                        m2s = range(m2_base, m2_base + MD // 2)
                        pys = {
                            m2: pspool.tile(
                                [P, CHUNK], f32, tag="ps", name=f"py{m2}"
                            )
                            for m2 in m2s
                        }
                        for k2 in range(MH):
                            for m2 in m2s:
                                nc.tensor.matmul(
                                    pys[m2][:, :tw],
                                    w2_sb[:, k2, ts(m2, P)],
                                    h_sb[:, k2, :tw],
                                    start=(k2 == 0),
                                    stop=(k2 == MH - 1),
                                )
                        for m2 in m2s:
                            y_sb = ypool.tile([P, CHUNK], bf16, tag="y")
                            nc.vector.tensor_copy(y_sb[:, :tw], pys[m2][:, :tw])
                            nc.sync.dma_start(
                                yt_d[ts(m2, P), off : off + tw], y_sb[:, :tw]
                            )
                else:
                    for m2 in range(MD):
                        # Split the very last m2-tile into column halves so
                        # its first CAST+DMA overlap the second half's
                        # matmuls instead of trailing the whole kernel.
                        if m2 == MD - 1:
                            halves = [(0, tw // 2), (tw // 2, tw)]
                        else:
                            halves = [(0, tw)]
                        for lo, hi in halves:
                            hw = hi - lo
                            py = pspool.tile([P, CHUNK], f32, tag="ps", name="py")
                            for k2 in range(MH):
                                nc.tensor.matmul(
                                    py[:, :hw],
                                    w2_sb[:, k2, ts(m2, P)],
                                    h_sb[:, k2, lo:hi],
                                    start=(k2 == 0),
                                    stop=(k2 == MH - 1),
                                )
                            y_sb = ypool.tile([P, CHUNK], bf16, tag="y")
                            nc.vector.tensor_copy(y_sb[:, :hw], py[:, :hw])
                            nc.sync.dma_start(
                                yt_d[ts(m2, P), off + lo : off + hi], y_sb[:, :hw]
                            )

            # PE warmup: junk matmuls on a memset tile run while the first
            # weight/activation DMAs land, so the HAM clock gate is already at
            # 8/8 when real matmuls start (~4.5us in).
            warm_sb = xpool.tile([P, P], bf16, name="warm")
            nc.vector.memset(warm_sb, 0.0)
            warm_ps = pspool.tile([P, P], f32, tag="ps", name="warm_ps")
            for _ in range(36):
                nc.tensor.matmul(warm_ps, warm_sb, warm_sb, start=True, stop=True)

            for idx, ci in enumerate(order):
                seg, off, tw = chunk_list[ci]
                w1_sb = w1_sbs[seg]
                h_sb = hpool.tile([P, MH, CHUNK], bf16, tag="h")
                for m in range(MH):
                    ph = pspool.tile([P, CHUNK], f32, tag="ps", name="ph")
                    for k in range(KD):
                        nc.tensor.matmul(
                            ph[:, :tw],
                            w1_sb[:, m, k],
                            xt_sbs[ci][:, k, :tw],
                            start=(k == 0),
                            stop=(k == KD - 1),
                        )
                    nc.scalar.activation(
                        h_sb[:, m, :tw],
                        ph[:, :tw],
                        mybir.ActivationFunctionType.Relu,
                        bias=b1_sb[:, seg * MH + m : seg * MH + m + 1],
                    )

                    if idx == 0 and m < 4:
                        # Insurance against DMA-arrival jitter during the
                        # ramp: dependency-free matmuls keep the HAM clock
                        # gate at 8/8 if a weight piece lands late.
                        for _ in range(4):
                            nc.tensor.matmul(
                                warm_ps, warm_sb, warm_sb, start=True, stop=True
                            )
                mm2_phase(seg, off, tw, h_sb, k2_outer=(idx < n_chunks - 1))
    nc.finalize()
    return nc


def _route(x, Wg):
    """Exact reference gating on host: top-2 of clean fp32 logits (jax
    tie-break: lower index first), softmax over the two picks."""
    logits = x @ Wg  # [B, E] fp32
    order = np.argsort(-logits, axis=1, kind="stable")[:, :TOP_K]  # [B, 2]
    top_vals = np.take_along_axis(logits, order, axis=1)
    ex = np.exp(top_vals - top_vals[:, :1])  # top_vals sorted desc -> max first
    gates = (ex / ex.sum(axis=1, keepdims=True)).astype(np.float32)  # [B, 2]
    return order, gates


def kernel(x, Wg, W1, b1, W2, b2):
    x = np.ascontiguousarray(np.asarray(x, dtype=np.float32))
    Wg = np.asarray(Wg, dtype=np.float32)
    W1 = np.asarray(W1, dtype=np.float32)
    b1 = np.asarray(b1, dtype=np.float32)
    W2 = np.asarray(W2, dtype=np.float32)
    b2 = np.asarray(b2, dtype=np.float32)

    order, gates = _route(x, Wg)

    # Dispatch: flatten (token, k) pairs, bucket by expert (stable => slot
    # order within an expert follows token order). Pair p belongs to token p//2.
    expert_flat = order.reshape(-1)  # [2B]
    gate_flat = gates.reshape(-1)  # [2B]
    perm = np.argsort(expert_flat, kind="stable")  # pairs grouped by expert
    counts = np.bincount(expert_flat, minlength=E)
    offs = np.concatenate(([0], np.cumsum(counts)))[:E]

    S1, S2, pieces = _plan_segments(counts)
    C = S1 + S2
    assert C <= 4864, f"unexpectedly imbalanced routing: {counts}"

    # Per-pair placement (core, column) for the combine step, and per-core
    # token lists for the dispatch.
    core_of_pair = np.empty(2 * B, dtype=np.int64)
    col_of_pair = np.empty(2 * B, dtype=np.int64)
    xT = np.ascontiguousarray(x.T)  # [D, B]
    bf16_w1 = [None] * E  # [MH*P, KD*P] permuted (m, p, ko, hl) layout
    bf16_w2 = [None] * E
    chunk_bounds = []  # (off, tw) in device column order
    off0 = 0
    for seg_len in (S1, S2):
        for tw in _chunk_sizes(seg_len):
            chunk_bounds.append((off0, tw))
            off0 += tw
    in_maps = []
    for core in range(E):
        xg = np.zeros((D, C), dtype=_BF16)
        b1ab = np.zeros((P, 2 * MH), dtype=np.float32)
        in_map = {"b1ab": b1ab}
        for seg, (e, tok_start, ln) in enumerate(pieces[core]):
            seg_off = 0 if seg == 0 else S1
            if ln:
                pair_idx = perm[offs[e] + tok_start : offs[e] + tok_start + ln]
                toks = pair_idx // 2
                xg[:, seg_off : seg_off + ln] = xT[:, toks].astype(_BF16)
                core_of_pair[pair_idx] = core
                col_of_pair[pair_idx] = seg_off + np.arange(ln)
            if bf16_w1[e] is None:
                bf16_w1[e] = np.ascontiguousarray(
                    W1[e]
                    .astype(_BF16)
                    .reshape(KD, P, MH, P)
                    .transpose(2, 1, 0, 3)
                    .reshape(MH * P, KD * P)
                )
                bf16_w2[e] = W2[e].astype(_BF16)
            s = "ab"[seg]
            in_map[f"w1{s}"] = bf16_w1[e]
            in_map[f"w2{s}"] = bf16_w2[e]
            b1ab[:, seg * MH : (seg + 1) * MH] = b1[e].reshape(MH, P).T
        # xt: chunk-major [p][ko][c-in-chunk] so every chunk DMA moves 8KB
        # contiguous per-partition runs.
        xtp = np.empty((P, KD * C), dtype=_BF16)
        for coff, tw in chunk_bounds:
            blk = xg[:, coff : coff + tw].reshape(KD, P, tw).transpose(1, 0, 2)
            xtp[:, KD * coff : KD * (coff + tw)] = blk.reshape(P, KD * tw)
        in_map["xt"] = xtp
        in_maps.append(in_map)

    nc = _build_program(S1, S2)

    from concourse.bass_utils import run_bass_kernel_spmd

    trace = os.environ.get("MOE_TRACE") == "1"
    kwargs = {}
    if trace:
        kwargs = dict(trace=True, trace_cores=list(range(E)))
    try:
        res = run_bass_kernel_spmd(nc, in_maps, core_ids=list(range(E)), **kwargs)
    except Exception:  # wedged accelerator: reset once and retry untraced
        try:
            import ctypes

            lib = ctypes.CDLL("/opt/axon/libaxon_pjrt.so")
            lib.axon_reset.restype = ctypes.c_int64
            lib.axon_reset()
        except OSError:
            pass
        res = run_bass_kernel_spmd(nc, in_maps, core_ids=list(range(E)))
    global LAST_RESULTS
    LAST_RESULTS = res

    Y = np.stack([r["yt"] for r in res.results])  # [E, D, C] bf16

    # Combine: pair p contributes gate_p * (y[:, col_p] + b2[e_p]) to token
    # p//2. Pairs of token b sit at flat positions 2b, 2b+1.
    cols = Y[core_of_pair, :, col_of_pair].astype(np.float32)  # [2B, D]
    weighted = (cols + b2[expert_flat]) * gate_flat[:, None]
    out = weighted[0::2] + weighted[1::2]
    return np.ascontiguousarray(out, dtype=np.float32)
